# revision 5
# baseline (speedup 1.0000x reference)
"""Trainium2 Bass kernel for EnhancedGNN (3x GCNConv + mean-pool + FC).

Self-contained: host-side sharding/layout prep + SPMD Bass/Tile program on 8
NeuronCores. See bottom for the `kernel(**inputs)` entry point.

Distribution strategy (sharding_hint: partition nodes + incident edges):
  - dst-nodes partitioned across cores (degree-balanced, 98 tiles x 128/core)
  - per layer: every core computes scaled features G = dinv * (X @ W) for the
    node range it can (replicated or shard), G rows live in HBM
  - edges bucketed by (dst-tile, src-block); DMA-gather of G[src] rows
    (int16 indices local to 25088-row blocks), one-hot S-matrix matmuls do
    the scatter-add into PSUM per dst tile
  - AllGather of X2^T / G3 / dinv between layers, AllReduce of pooled sums
"""

import math
import os
import sys

import numpy as np

for _p in ("/opt/trn_rl_repo", "/root/.axon_site", "/root/.axon_site/_ro/pypackages"):
    if os.path.isdir(_p) and _p not in sys.path:
        sys.path.append(_p)

P = 128


def cdiv(a, b):
    return -(-a // b)


class Cfg:
    def __init__(self, n_nodes, n_edges, nc, tiles_pc, grp, nblk, n_graphs):
        self.N = n_nodes
        self.E = n_edges
        self.NC = nc
        self.T = tiles_pc
        self.GRP = grp
        self.NBLK = nblk
        self.G = n_graphs
        self.NPC = self.T * P
        self.TOTAL = self.NC * self.NPC
        self.BLK = self.TOTAL // self.NBLK
        assert self.T % self.GRP == 0
        assert self.TOTAL % self.NBLK == 0
        assert self.BLK <= 32768
        assert self.N % self.NC == 0
        assert self.N // self.NC <= self.NPC
        # filled by host_prep:
        self.CHT = None  # chunks per (tile, block) bucket
        self.SC = None  # degree-slot cap per node
        self.F = (64, 64, 128, 64)  # F0(in), F1, F2, F3


FULL_CFG = dict(n_nodes=100000, n_edges=3200000, nc=8, tiles_pc=98, grp=7,
                nblk=4, n_graphs=64)


# --------------------------------------------------------------------------
# Host-side prep: node assignment, edge bucketing, layout arrays.
# Pure index manipulation / data layout (the sharding step); all FLOPs of the
# reference computation happen on device.
# --------------------------------------------------------------------------

def host_prep(x, src, dst, edge_weight, batch, W1, b1, W2, b2, W3, b3, Wfc,
              bfc, cfg: Cfg):
    N, E, NC, T = cfg.N, cfg.E, cfg.NC, cfg.T
    NPC, TOTAL, NBLK, BLK, GRP = cfg.NPC, cfg.TOTAL, cfg.NBLK, cfg.BLK, cfg.GRP
    F0 = cfg.F[0]
    x = np.ascontiguousarray(np.asarray(x, np.float32))
    src = np.asarray(src).astype(np.int64)
    dst = np.asarray(dst).astype(np.int64)
    ew = np.asarray(edge_weight, np.float32)
    batch = np.asarray(batch).astype(np.int64)

    # ---- node -> (core, tile, p) assignment, degree balanced ----
    degc = np.bincount(dst, minlength=N)
    order = np.argsort(-degc, kind="stable")
    ranks = np.arange(N)
    core_of = np.empty(N, np.int64)
    rank_in_core = np.empty(N, np.int64)
    core_of[order] = ranks % NC
    rank_in_core[order] = ranks // NC
    row = rank_in_core // T
    col = rank_in_core % T
    tile = np.where(row % 2 == 0, col, T - 1 - col)
    p_in_tile = row
    assert p_in_tile.max() < P
    local = tile * P + p_in_tile
    gperm = core_of * NPC + local  # global permuted node id (tile-major)
    # G-row id (DMA-friendly group layout: groups of GRP tiles, row-major by p)
    grp_i = tile // GRP
    j_i = tile % GRP
    grow = core_of * NPC + grp_i * (GRP * P) + p_in_tile * GRP + j_i

    # ---- edge bucketing by (dst core, dst tile, src block) ----
    # self-loops (weight 1.0) are appended as ordinary edges; their
    # norm dinv[d]*1*dinv[d] falls out of the same gather+S-matmul path.
    loop = np.arange(N, dtype=np.int64)
    src_f = np.concatenate([src, loop])
    dst_f = np.concatenate([dst, loop])
    ew_f = np.concatenate([ew, np.ones(N, np.float32)])
    Ef = E + N
    e_core = core_of[dst_f]
    e_tile = tile[dst_f]
    e_p = p_in_tile[dst_f]
    e_grow = grow[src_f]
    e_B = e_grow // BLK
    e_lidx = (e_grow % BLK).astype(np.int64)
    key = (e_core * T + e_tile) * NBLK + e_B
    si = np.argsort(key, kind="stable")
    key_s = key[si]
    nbuck = NC * T * NBLK
    bc = np.bincount(key_s, minlength=nbuck)
    CHT = max(1, cdiv(int(bc.max()), P))
    CAP = CHT * P
    CAP16 = CAP // 16
    cfg.CHT = CHT
    starts = np.zeros(nbuck + 1, np.int64)
    np.cumsum(bc, out=starts[1:])
    slot = np.arange(Ef) - starts[key_s]

    core_b = key_s // (T * NBLK)
    buck = key_s % (T * NBLK)

    idx_arr = np.empty((NC, T * NBLK, CAP), np.int16)
    pad_idx = ((np.arange(CAP, dtype=np.int64) * 977) % BLK).astype(np.int16)
    idx_arr[:] = pad_idx[None, None, :]
    idx_arr[core_b, buck, slot] = e_lidx[si].astype(np.int16)

    dstf = np.full((NC, T * NBLK * CHT, P), -1.0, np.float32)
    wf = np.zeros((NC, T * NBLK * CHT, P), np.float32)
    colb = buck * CHT + slot // P
    pp = slot % P
    dstf[core_b, colb, pp] = e_p[si].astype(np.float32)
    wf[core_b, colb, pp] = ew_f[si]

    # 16-wrap the indices: logical i -> [i % 16, i // 16], replicate x8 rows
    idx16 = idx_arr.reshape(NC, T * NBLK, CAP16, 16).transpose(0, 1, 3, 2)
    idx16 = np.tile(idx16, (1, 1, 8, 1))  # [NC, buck, 128, CAP16]
    # DRAM layout for contiguous per-segment loads:
    # [NC, NGRP, 128, GRP*NBLK*CAP16]
    GRPc = cfg.GRP
    NGRP = T // GRPc
    idx16 = idx16.reshape(NC, NGRP, GRPc, NBLK, P, CAP16)
    idx16 = np.ascontiguousarray(idx16.transpose(0, 1, 4, 2, 3, 5)).reshape(
        NC, NGRP, P, GRPc * NBLK * CAP16)

    dstf_t = np.ascontiguousarray(dstf.transpose(0, 2, 1))  # [NC, 128, cols]
    wf_t = np.ascontiguousarray(wf.transpose(0, 2, 1))

    # ---- degree slots (for deg = sum of w per dst node) ----
    si2 = np.argsort(dst, kind="stable")
    d2 = dst[si2]
    bc2 = np.bincount(d2, minlength=N)
    SC = max(1, int(bc2.max()))
    cfg.SC = SC
    st2 = np.zeros(N + 1, np.int64)
    np.cumsum(bc2, out=st2[1:])
    k2 = np.arange(E) - st2[d2]
    wslot = np.zeros((NC, T * SC, P), np.float32)
    wslot[core_of[d2], tile[d2] * SC + k2, p_in_tile[d2]] = ew[si2]
    wslot_t = np.ascontiguousarray(wslot.transpose(0, 2, 1))  # [NC, 128, T*SC]

    # ---- batch one-hot source values (permuted), pad -> -1 ----
    batchf = np.full((NC, P, T), -1.0, np.float32)
    batchf[core_of, p_in_tile, tile] = batch.astype(np.float32)

    # ---- features transposed into permuted order ----
    xT = np.zeros((F0, TOTAL), np.float32)
    xT[:, gperm] = x.T

    # ---- constants ----
    iota = np.tile(np.arange(P, dtype=np.float32)[None, :], (P, 1))
    ident = np.eye(P, dtype=np.float32)
    ones = np.ones((P, 1), np.float32)

    per_core = []
    for c in range(NC):
        m = {
            "xT": xT,
            "idx16": np.ascontiguousarray(idx16[c]),
            "dstf": dstf_t[c],
            "wf": wf_t[c],
            "wslot": wslot_t[c],
            "batchf": np.ascontiguousarray(batchf[c]),
            "iota": iota,
            "ident": ident,
            "ones": ones,
            "W1": np.asarray(W1, np.float32),
            "W2": np.asarray(W2, np.float32),
            "W3": np.asarray(W3, np.float32),
            "Wfc": np.asarray(Wfc, np.float32).reshape(cfg.F[3], 1),
            "b1r": np.tile(np.asarray(b1, np.float32)[None, :], (P, 1)),
            "b2r": np.tile(np.asarray(b2, np.float32)[None, :], (P, 1)),
            "b3r": np.tile(np.asarray(b3, np.float32)[None, :], (P, 1)),
            "bfcr": np.full((64, 1), np.float32(np.asarray(bfc).reshape(-1)[0])),
        }
        per_core.append(m)
    return per_core


# --------------------------------------------------------------------------
# Bass/Tile SPMD program
# --------------------------------------------------------------------------

def build_program(cfg: Cfg):
    import concourse.bacc as bacc
    import concourse.bass as bass
    import concourse.mybir as mybir
    import concourse.tile as tile

    dt = mybir.dt
    f32 = dt.float32
    Alu = mybir.AluOpType
    Act = mybir.ActivationFunctionType

    NC, T, GRP, NBLK = cfg.NC, cfg.T, cfg.GRP, cfg.NBLK
    NPC, TOTAL, BLK = cfg.NPC, cfg.TOTAL, cfg.BLK
    CHT, SC, G = cfg.CHT, cfg.SC, cfg.G
    CAP = CHT * P
    CAP16 = CAP // 16
    F0, F1, F2, F3 = cfg.F
    NGRP = T // GRP
    GR = GRP * P  # rows per G write group
    FMX = max(F1, F2, F3)

    nc = bacc.Bacc("TRN2", target_bir_lowering=False, debug=False,
                   enable_asserts=False, num_devices=NC)

    def inp(name, shape, dtype=f32):
        return nc.dram_tensor(name, list(shape), dtype, kind="ExternalInput")

    xT = inp("xT", (F0, TOTAL))
    idx16 = inp("idx16", (NGRP, P, GRP * NBLK * CAP16), dt.int16)
    dstf = inp("dstf", (P, T * NBLK * CHT))
    wf = inp("wf", (P, T * NBLK * CHT))
    wslot = inp("wslot", (P, T * SC))
    batchf = inp("batchf", (P, T))
    iota_in = inp("iota", (P, P))
    ident_in = inp("ident", (P, P))
    ones_in = inp("ones", (P, 1))
    W_in = [inp("W1", (F0, F1)), inp("W2", (F1, F2)), inp("W3", (F2, F3))]
    Wfc_in = inp("Wfc", (F3, 1))
    b_in = [inp("b1r", (P, F1)), inp("b2r", (P, F2)), inp("b3r", (P, F3))]
    bfc_in = inp("bfcr", (64, 1))
    out_t = nc.dram_tensor("out", [64, 1], f32, kind="ExternalOutput")

    rg = [list(range(NC))]

    with tile.TileContext(nc) as tc:
        import contextlib
        ctx = contextlib.ExitStack()
        with ctx:
            dram = ctx.enter_context(tc.tile_pool(name="dram", bufs=1, space="DRAM"))
            pers = ctx.enter_context(tc.tile_pool(name="pers", bufs=1))
            sb2 = ctx.enter_context(tc.tile_pool(name="sb2", bufs=2))
            sb3 = ctx.enter_context(tc.tile_pool(name="sb3", bufs=3))
            spool = ctx.enter_context(tc.tile_pool(name="spool", bufs=6))
            gpool = ctx.enter_context(tc.tile_pool(name="gpool", bufs=3))
            gemm_ps = ctx.enter_context(tc.tile_pool(name="gemm_ps", bufs=2, space="PSUM"))
            agg_ps = ctx.enter_context(tc.tile_pool(name="agg_ps", bufs=4, space="PSUM"))
            tp_ps = ctx.enter_context(tc.tile_pool(name="tp_ps", bufs=2, space="PSUM"))

            # ---------- DRAM intermediates ----------
            G_blk = [[dram.tile([BLK, f], f32, name=f"G{li}_{b}")
                      for b in range(NBLK)] for li, f in ((0, F1), (1, F2))]
            G3_shard = dram.tile([NPC, F3], f32, name="G3_shard")
            G3_full = dram.tile([TOTAL, F3], f32, name="G3_full", addr_space="Shared")
            X2T_shard = dram.tile([F1, NPC], f32, name="X2T_shard")
            X2T_full = dram.tile([NC * F1, NPC], f32, name="X2T_full", addr_space="Shared")
            dinv_shard = dram.tile([T, P], f32, name="dinv_shard")
            dinv_full = dram.tile([NC * T, P], f32, name="dinv_full", addr_space="Shared")
            pool_in = dram.tile([64, F3 + 1], f32, name="pool_in")
            pool_out = dram.tile([64, F3 + 1], f32, name="pool_out", addr_space="Shared")

            # ---------- constants ----------
            iota_sb = pers.tile([P, P], f32, name="iota_sb")
            ident_sb = pers.tile([P, P], f32, name="ident_sb")
            ones_sb = pers.tile([P, 1], f32, name="ones_sb")
            nc.sync.dma_start(iota_sb[:], iota_in[:])
            nc.sync.dma_start(ident_sb[:], ident_in[:])
            nc.sync.dma_start(ones_sb[:], ones_in[:])
            W_sb = []
            for li, w in enumerate(W_in):
                t_ = pers.tile(list(w.shape), f32, name=f"W{li + 1}_sb")
                nc.sync.dma_start(t_[:], w[:])
                W_sb.append(t_)
            Wfc_sb = pers.tile([F3, 1], f32, name="Wfc_sb")
            nc.sync.dma_start(Wfc_sb[:], Wfc_in[:])
            b_sb = []
            for li, b in enumerate(b_in):
                t_ = pers.tile(list(b.shape), f32, name=f"b{li + 1}_sb")
                nc.sync.dma_start(t_[:], b[:])
                b_sb.append(t_)
            bfc_sb = pers.tile([64, 1], f32, name="bfc_sb")
            nc.sync.dma_start(bfc_sb[:], bfc_in[:])
            batchf_sb = pers.tile([P, T], f32, name="batchf_sb")
            nc.sync.dma_start(batchf_sb[:], batchf[:])
            dstf_sb = pers.tile([P, T * NBLK * CHT], f32, name="dstf_sb")
            wf_sb = pers.tile([P, T * NBLK * CHT], f32, name="wf_sb")
            nc.sync.dma_start(dstf_sb[:], dstf[:])
            nc.sync.dma_start(wf_sb[:], wf[:])

            # ---------- degree -> dinv ----------
            dinv_self = pers.tile([P, T], f32, name="dinv_self")
            with tc.tile_pool(name="wslp", bufs=1) as wslp:
                wsl_sb = wslp.tile([P, T * SC], f32, name="wsl_sb")
                nc.sync.dma_start(wsl_sb[:], wslot[:])
                deg_sb = sb2.tile([P, T], f32, name="deg_sb", tag="deg")
                sq_sb = sb2.tile([P, T], f32, name="sq_sb", tag="sq")
                for t in range(T):
                    nc.vector.tensor_reduce(deg_sb[:, t:t + 1],
                                            wsl_sb[:, t * SC:(t + 1) * SC],
                                            mybir.AxisListType.X, Alu.add)
                # sqrt(deg + 1) then 1/x
                nc.scalar.activation(sq_sb[:], deg_sb[:], Act.Sqrt, bias=1.0)
                nc.vector.reciprocal(dinv_self[:], sq_sb[:])
            # publish dinv: transpose [128,T] -> [T,128] -> dram -> AllGather
            dps = tp_ps.tile([T, P], f32, name="dps", tag="tp")
            nc.tensor.transpose(dps[:], dinv_self[:], ident_sb[:])
            dtr = sb2.tile([T, P], f32, name="dtr", tag="dtr")
            nc.vector.tensor_copy(dtr[:], dps[:])
            nc.sync.dma_start(dinv_shard[:], dtr[:])
            nc.gpsimd.collective_compute(
                "AllGather", Alu.bypass, ins=[dinv_shard.opt()],
                outs=[dinv_full.opt()], replica_groups=rg)
            dinvF = pers.tile([P, NC * T], f32, name="dinvF")
            for c in range(NC):
                dl = sb3.tile([T, P], f32, name="dl", tag="dl")
                nc.sync.dma_start(dl[:], dinv_full[c * T:(c + 1) * T, :])
                dq = tp_ps.tile([P, T], f32, name="dq", tag="tp")
                nc.tensor.transpose(dq[:], dl[:], ident_sb[:T, :T])
                nc.vector.tensor_copy(dinvF[:, c * T:(c + 1) * T], dq[:])

            x2t_stage_tag = "x2t_stage"
            pool_sb = pers.tile([64, F3 + 1], f32, name="pool_sb")
            nc.vector.memset(pool_sb[:], 0.0)

            # ================= helpers =================

            def gemm_full(li):
                """G_blk[li][*] = dinvF * (X @ W) for all nodes (replicated).
                li==0 streams xT; li==1 streams X2T_full rows."""
                F_out = cfg.F[li + 1]
                for c in range(NC):
                    for g in range(NGRP):
                        piece = gpool.tile([cfg.F[li], GR], f32, name="xp", tag="lhs")
                        if li == 0:
                            src_ap = xT[:, c * NPC + g * GR: c * NPC + (g + 1) * GR]
                        else:
                            src_ap = X2T_full[c * F1:(c + 1) * F1,
                                              g * GR:(g + 1) * GR]
                        nc.sync.dma_start(piece[:], src_ap)
                        stage = sb3.tile([P, GRP * F_out], f32, name="gstage",
                                         tag="gstage")
                        for j in range(GRP):
                            t = g * GRP + j
                            ps = gemm_ps.tile([P, F_out], f32, name="psf", tag="gps")
                            nc.tensor.matmul(ps[:], lhsT=piece[:, j * P:(j + 1) * P],
                                             rhs=W_sb[li][:], start=True, stop=True)
                            nc.vector.tensor_scalar(
                                stage[:, j * F_out:(j + 1) * F_out], ps[:],
                                dinvF[:, c * T + t: c * T + t + 1], None, Alu.mult)
                        # store stage -> G rows [(c%cpb)*NPC + g*GR ...] of block
                        b = (c * NPC) // BLK
                        roff = (c * NPC) % BLK + g * GR
                        gt = G_blk[li][b]
                        dst_ap = gt[roff:roff + GR, :].rearrange(
                            "(p j) f -> p j f", j=GRP)
                        nc.sync.dma_start(dst_ap, stage[:].rearrange(
                            "p (j f) -> p j f", j=GRP))

            def agg_layer(li):
                """S_t accumulation + epilogue for layer li; produces next X
                (fused transposes/GEMMs for the next layer's self-G) and, for
                li==2, the pooled partial sums."""
                abl = os.environ.get("K_ABL", "")
                F_out = cfg.F[li + 1]
                if li == 2:
                    if abl == "g3local":
                        G3_local = dram.tile([TOTAL, F3], f32, name="G3_local")
                        nc.sync.dma_start(G3_local[:], G3_full[:])
                        g_src = [G3_local[b * BLK:(b + 1) * BLK, :]
                                 for b in range(NBLK)]
                    else:
                        g_src = [G3_full[b * BLK:(b + 1) * BLK, :]
                                 for b in range(NBLK)]
                else:
                    g_src = [t_[:] for t_ in G_blk[li]]
                skip_mm = (abl == f"agg{li + 1}_nomm")
                skip_g = (abl == f"agg{li + 1}_nogather")
                first_gb = [None]
                for g in range(NGRP):
                    idxs_t = gpool.tile([P, GRP * NBLK * CAP16], dt.int16,
                                        name="idxs_t", tag="idx")
                    nc.sync.dma_start(idxs_t[:], idx16[g, :, :])
                    for j in range(GRP):
                        t = g * GRP + j
                        aps = agg_ps.tile([P, F_out], f32, name="aps", tag="aps")
                        nmm = NBLK * CHT
                        for b in range(NBLK):
                            if skip_g and (t > 0 or b > 0):
                                gb = first_gb[0]
                            else:
                                gb = gpool.tile([P, CHT * F_out], f32, name="gb",
                                                tag="gb")
                                nc.gpsimd.dma_gather(
                                    gb[:].rearrange("p (k f) -> p k f", k=CHT),
                                    g_src[b],
                                    idxs_t[:, (j * NBLK + b) * CAP16:
                                           (j * NBLK + b + 1) * CAP16],
                                    CAP, CAP, F_out, single_packet=False)
                                if skip_g:
                                    first_gb[0] = gb
                            for k in range(CHT):
                                kk = b * CHT + k
                                col = (t * NBLK + b) * CHT + k
                                if skip_mm and kk > 0:
                                    continue
                                S = spool.tile([P, P], f32, name="S", tag="S")
                                nc.vector.tensor_scalar(
                                    S[:], iota_sb[:], dstf_sb[:, col:col + 1],
                                    wf_sb[:, col:col + 1], Alu.is_equal,
                                    op1=Alu.mult)
                                nc.tensor.matmul(
                                    aps[:], lhsT=S[:],
                                    rhs=gb[:, k * F_out:(k + 1) * F_out],
                                    start=(kk == 0),
                                    stop=(kk == nmm - 1 or skip_mm))
                        agg_epilogue(li, t, aps, F_out)

            def agg_epilogue(li, t, aps, F_out):
                # X = relu(dinv * S_agg + b)
                ts2 = sb3.tile([P, F_out], f32, name="ts2", tag="ep2")
                nc.vector.tensor_scalar(ts2[:], aps[:], dinv_self[:, t:t + 1],
                                        None, Alu.mult)
                ts3 = sb3.tile([P, F_out], f32, name="ts3", tag="ep3")
                nc.vector.tensor_tensor(out=ts3[:], in0=ts2[:], in1=b_sb[li][:],
                                        op=Alu.add)
                X = sb3.tile([P, F_out], f32, name="X", tag="epx")
                nc.scalar.activation(X[:], ts3[:], Act.Relu)
                g = t // GRP
                j = t % GRP
                if li == 0:
                    # transpose -> stage -> X2T_shard (AllGather input)
                    tp = tp_ps.tile([F_out, P], f32, name="tpx", tag="tp")
                    nc.tensor.transpose(tp[:], X[:], ident_sb[:])
                    st = _stage_for(li, g)
                    nc.vector.tensor_copy(st[:, j * P:(j + 1) * P], tp[:])
                    if j == GRP - 1:
                        nc.sync.dma_start(
                            X2T_shard[:, g * GR:(g + 1) * GR], st[:])
                elif li == 1:
                    # fused: X3^T -> G3 = dinv * (X3 @ W3) -> G3_shard stage
                    tp = tp_ps.tile([F_out, P], f32, name="tpx3", tag="tp")
                    nc.tensor.transpose(tp[:], X[:], ident_sb[:])
                    xt_piece = spool.tile([F_out, P], f32, name="x3tp", tag="x2tp")
                    nc.vector.tensor_copy(xt_piece[:], tp[:])
                    ps3 = gemm_ps.tile([P, F3], f32, name="ps3", tag="gps")
                    nc.tensor.matmul(ps3[:], lhsT=xt_piece[:], rhs=W_sb[2][:],
                                     start=True, stop=True)
                    st = _stage_for(li, g)
                    nc.vector.tensor_scalar(
                        st[:, j * F3:(j + 1) * F3], ps3[:],
                        dinv_self[:, t:t + 1], None, Alu.mult)
                    if j == GRP - 1:
                        dst_ap = G3_shard[g * GR:(g + 1) * GR, :].rearrange(
                            "(p j) f -> p j f", j=GRP)
                        nc.sync.dma_start(dst_ap, st[:].rearrange(
                            "p (j f) -> p j f", j=GRP))
                else:
                    # pooling: B_t one-hot over graphs, matmul partials
                    Bt = spool.tile([P, 64], f32, name="Bt", tag="Bt")
                    nc.vector.tensor_scalar(Bt[:], iota_sb[:, :64],
                                            batchf_sb[:, t:t + 1], None,
                                            Alu.is_equal)
                    pp = tp_ps.tile([64, F3 + 1], f32, name="ppool", tag="tp")
                    nc.tensor.matmul(pp[:, :F3], lhsT=Bt[:], rhs=X[:],
                                     start=True, stop=True)
                    nc.tensor.matmul(pp[:, F3:F3 + 1], lhsT=Bt[:],
                                     rhs=ones_sb[:], start=True, stop=True)
                    nc.vector.tensor_tensor(out=pool_sb[:], in0=pool_sb[:],
                                            in1=pp[:], op=Alu.add)

            _stages = {}

            def _stage_for(li, g):
                key = (li, g)
                if key not in _stages:
                    w = GRP * (P if li == 0 else F3)
                    _stages[key] = sb3.tile([F1 if li == 0 else P, w], f32,
                                            name=f"st{li}_{g}", tag=x2t_stage_tag)
                return _stages[key]

            # ================= execution =================
            cut = os.environ.get("K_CUT", "")

            def _cut(stage):
                return cut and cut == stage

            while True:
                if _cut("deg"):
                    break
                gemm_full(0)
                if _cut("gemm1"):
                    break
                agg_layer(0)
                if _cut("agg1"):
                    break
                nc.gpsimd.collective_compute(
                    "AllGather", Alu.bypass, ins=[X2T_shard.opt()],
                    outs=[X2T_full.opt()], replica_groups=rg)
                gemm_full(1)
                if _cut("gemm2"):
                    break
                agg_layer(1)
                if _cut("agg2"):
                    break
                nc.gpsimd.collective_compute(
                    "AllGather", Alu.bypass, ins=[G3_shard.opt()],
                    outs=[G3_full.opt()], replica_groups=rg)
                agg_layer(2)
                if _cut("agg3"):
                    break

                # ---- pool + FC ----
                nc.sync.dma_start(pool_in[:], pool_sb[:])
                nc.gpsimd.collective_compute(
                    "AllReduce", Alu.add, ins=[pool_in.opt()],
                    outs=[pool_out.opt()], replica_groups=rg)
                pr = sb2.tile([64, F3 + 1], f32, name="pr", tag="pr")
                nc.sync.dma_start(pr[:], pool_out[:])
                cmx = sb2.tile([64, 1], f32, name="cmx", tag="cmx")
                nc.vector.tensor_scalar(cmx[:], pr[:, F3:F3 + 1], 1.0, None,
                                        Alu.max)
                cinv = sb2.tile([64, 1], f32, name="cinv", tag="cinv")
                nc.vector.reciprocal(cinv[:], cmx[:])
                tsp = tp_ps.tile([F3, 64], f32, name="tsp", tag="tp")
                nc.tensor.transpose(tsp[:], pr[:, :F3], ident_sb[:64, :64])
                sT = sb2.tile([F3, 64], f32, name="sT", tag="sT")
                nc.vector.tensor_copy(sT[:], tsp[:])
                fps = tp_ps.tile([64, 1], f32, name="fps", tag="tp")
                nc.tensor.matmul(fps[:], lhsT=sT[:], rhs=Wfc_sb[:], start=True,
                                 stop=True)
                res = sb2.tile([64, 1], f32, name="res", tag="res")
                nc.vector.tensor_scalar(res[:], fps[:], cinv[:], bfc_sb[:],
                                        Alu.mult, op1=Alu.add)
                nc.sync.dma_start(out_t[:], res[:])
                break

    nc.compile()
    return nc


# --------------------------------------------------------------------------
# Entry point
# --------------------------------------------------------------------------

_PROGRAM_CACHE = {}


def kernel(x, src, dst, edge_weight, batch, W1, b1, W2, b2, W3, b3, Wfc, bfc):
    from concourse.bass_utils import run_bass_kernel_spmd

    cfg = Cfg(**FULL_CFG)
    per_core = host_prep(x, src, dst, edge_weight, batch, W1, b1, W2, b2, W3,
                         b3, Wfc, bfc, cfg)
    key = (cfg.CHT, cfg.SC)
    if key not in _PROGRAM_CACHE:
        _PROGRAM_CACHE[key] = build_program(cfg)
    nc = _PROGRAM_CACHE[key]
    res = run_bass_kernel_spmd(nc, per_core, list(range(cfg.NC)))
    out = np.asarray(res.results[0]["out"], np.float32).reshape(cfg.G, 1)
    return out

